# revision 20
# baseline (speedup 1.0000x reference)
"""Trainium2 Bass kernel for GRNNTransformSimple (bottom-up binary-tree GRNN).

Computation (per jet): heap-layout complete binary tree, DEPTH=14.
  u_k   = relu(contents_k @ Wu + bu)                         (all nodes)
  emb_k = u_k                                                (leaves)
  emb_k = relu(hL @ Wh[:64] + hR @ Wh[64:128] + u_k @ Wh[128:] + bh)  (inner)
Output: root emb, [B, 64].

Mapping (8 NeuronCores, data-parallel over B=128 jets, 16 jets/core):
 - 2 jets packed per 128 SBUF partitions (jet A on partitions 0-63, jet B on
   64-127) with block-diagonal weights -> all engines run 128 partitions wide.
 - fc_u biases folded into the matmul via a constant-one input row (K=18).
 - The "irregular" child gather is regular for arange children: children of
   level-i node j are nodes 2j, 2j+1 of level i+1, i.e. a stride-2 column
   slice of the level-(i+1) embedding buffer.

v2 notes (PE-floor oriented; matmul cost on TRN2 is output-columns at
~0.44 ns/col regardless of K, so fc_u is psum-write-bound and fc_h is
ingress-bound; total mm floor ~144us/core):
 - One shared 4-deep PSUM pool so act engines can lag the PE by up to 4
   tiles during the act-heavy u-phases without stalling the PE.
 - Strip-major u-phases + stationary-major fc_h groups + an LDWEIGHTS dedup
   pass that sees through NoOp/semaphore instructions: minimal weight loads.
 - Greedy ns-balanced assignment of relu activations to ScalarE/VectorE.
 - Levels 9..1 run as two independent 4-pair chains, interleaved per level,
   so one chain's matmuls cover the other's act->matmul latency.
"""

import os
import sys

sys.path.insert(0, "/opt/trn_rl_repo")

import ml_dtypes
import numpy as np

DEPTH = 14
B = 128
F = 8
H = 64
N_NODES = 2**DEPTH - 1  # 16383
N_INNER = 2 ** (DEPTH - 1) - 1  # 8191
N_CORES = 8
JPC = 16  # jets per core
NPAIR = 8  # jet pairs per core

BF16 = ml_dtypes.bfloat16

# u_stream layout per pair (columns): levels 10,11,12 inner nodes in heap
# order, then all leaves in heap order.
UB10, UB11, UB12 = 0, 1024, 3072  # level bases inside u_stream
ULEAF = 7168
USTREAM = 15360  # 1024 + 2048 + 4096 + 8192
NGRP = 15  # 15 groups x 1024 cols
# u_top: levels 0..9, column order [level][pair][node]
UTOP_COLS = 8184  # 8 * 1023
UTOP_PAD = 8192


def _np_reference(contents, children, Wu, bu, Wh, bh):
    emb = None
    for i in range(DEPTH - 1, -1, -1):
        off, n = 2**i - 1, 2**i
        u = np.maximum(contents[:, off : off + n] @ Wu + bu, 0)
        if emb is None:
            emb = u
        else:
            ch = children[off : off + n] - 2 * off
            hL = emb[:, ch[:, 0]]
            hR = emb[:, ch[:, 1]]
            emb = np.maximum(
                hL @ Wh[:H] + hR @ Wh[H : 2 * H] + u @ Wh[2 * H :] + bh, 0
            )
    return emb.reshape(emb.shape[0], -1).astype(np.float32)


def _prep_core_inputs(contents):
    """contents: [16, 16383, 8] f32 for one core.
    Returns dict of per-core device input arrays."""
    c4 = np.zeros((NPAIR, 128, 4096), dtype=BF16)
    big_T = np.ascontiguousarray(
        np.transpose(contents[:, 1023:16383, :], (0, 2, 1))
    )  # [16, 8, 15360]
    for p in range(NPAIR):
        S = np.empty((18, USTREAM), dtype=np.float32)
        S[0:8] = big_T[2 * p]
        S[8] = 1.0
        S[9:17] = big_T[2 * p + 1]
        S[17] = 1.0
        Sb = S.astype(BF16)
        for g in range(NGRP):
            t = g % 4
            cc = 1024 * (g // 4)
            c4[p, 32 * t : 32 * t + 18, cc : cc + 1024] = Sb[
                :, 1024 * g : 1024 * (g + 1)
            ]

    # u_top stream: [level][pair][node]
    node_idx = np.concatenate(
        [np.arange(2**i - 1, 2**i - 1 + 2**i) for i in range(10)]
    )  # [1023] heap indices, level-major
    # per level block repeated per pair
    tops = np.empty((18, UTOP_COLS), dtype=np.float32)
    colptr = 0
    cT = np.transpose(contents, (0, 2, 1))  # [16, 8, 16383]
    for i in range(10):
        off, n = 2**i - 1, 2**i
        for p in range(NPAIR):
            tops[0:8, colptr : colptr + n] = cT[2 * p][:, off : off + n]
            tops[8, colptr : colptr + n] = 1.0
            tops[9:17, colptr : colptr + n] = cT[2 * p + 1][:, off : off + n]
            tops[17, colptr : colptr + n] = 1.0
            colptr += n
    assert colptr == UTOP_COLS
    ctop = np.zeros((128, 2048), dtype=BF16)
    tb = np.zeros((18, UTOP_PAD), dtype=BF16)
    tb[:, :UTOP_COLS] = tops.astype(BF16)
    for g in range(8):
        t = g % 4
        cc = 1024 * (g // 4)
        ctop[32 * t : 32 * t + 18, cc : cc + 1024] = tb[:, 1024 * g : 1024 * (g + 1)]
    return {"c4": c4, "ctop": ctop}


def _prep_weights(Wu, bu, Wh, bh):
    wu2 = np.zeros((18, 128), dtype=np.float32)
    wu2[0:8, 0:64] = Wu
    wu2[8, 0:64] = bu
    wu2[9:17, 64:128] = Wu
    wu2[17, 64:128] = bu
    # Four full-K stationaries (one per 32-row strip): rows outside the
    # strip are zero so the other strips' data in the moving columns
    # contributes nothing. Full-K keeps the mm "dense" (128x128) from the
    # DVFS governor's perspective.
    wu_dram = np.zeros((4, 128, 128), dtype=BF16)
    for t in range(4):
        wu_dram[t, 32 * t : 32 * t + 18, :] = wu2.astype(BF16)
    wu_dram = wu_dram.transpose(1, 0, 2).reshape(128, 512)

    def blockdiag(Wx):
        out = np.zeros((128, 128), dtype=np.float32)
        out[0:64, 0:64] = Wx
        out[64:128, 64:128] = Wx
        return out.astype(BF16)

    whl = blockdiag(Wh[0:H])
    whr = blockdiag(Wh[H : 2 * H])
    whu = blockdiag(Wh[2 * H : 3 * H])
    bh2 = np.concatenate([bh, bh]).astype(np.float32).reshape(128, 1)
    return {"wu": wu_dram, "whl": whl, "whr": whr, "whu": whu, "bh2": bh2}


def _dedup_ldweights(nc):
    """Delete an LDWEIGHTS whose signature matches the previous PE weight
    load when only instructions that cannot disturb the stationary operand
    (MATMULs, NoOps, semaphore ops) execute in between: the PE keeps the
    stationary resident, so load-once-matmul-many is safe. Sync info of
    deleted loads is merged into the following PE instruction."""
    n_del = 0
    transparent = ("InstMatmult", "InstNoOp", "InstEventSemaphore")
    for f in nc.m.functions:
        for bb in f.blocks:
            last_sig = None
            pending_sync = None
            out = []
            for inst in bb.instructions:
                tn = type(inst).__name__
                if str(getattr(inst, "engine", "")) == "EngineType.PE":
                    if tn == "InstLdweights":
                        a = inst.ins[0]
                        sig = (
                            getattr(a, "memref", None),
                            getattr(a, "offset", None),
                            str(getattr(a, "ap", None)),
                            str(inst.tile_position),
                            str(inst.tile_size),
                            str(inst.perf_mode),
                            str(inst.is_transpose),
                        )
                        if sig == last_sig:
                            n_del += 1
                            si = inst.sync_info
                            if si is not None and (si.on_wait or si.on_update):
                                if pending_sync is None:
                                    pending_sync = ([], [])
                                pending_sync[0].extend(si.on_wait)
                                pending_sync[1].extend(si.on_update)
                            continue  # drop this instruction
                        last_sig = sig
                    elif tn not in transparent:
                        last_sig = None  # anything else on PE invalidates
                    if pending_sync is not None:
                        si = inst.sync_info
                        if si is None:
                            import concourse.mybir as mybir

                            inst.sync_info = mybir.SyncInfo(
                                on_wait=list(pending_sync[0]),
                                on_update=list(pending_sync[1]),
                            )
                        else:
                            si.on_wait[:0] = pending_sync[0]
                            si.on_update.extend(pending_sync[1])
                        pending_sync = None
                out.append(inst)
            assert pending_sync is None, "dangling sync from deleted trailing LDW"
            bb.instructions.clear()
            for i in out:
                bb.add_instruction(i)
    return n_del


def _split_sync_waits(nc, mybir, max_waits=1):
    """This container's walrus only accepts 1 sync-wait per instruction;
    move excess waits onto preceding same-engine NoOps."""
    for f in nc.m.functions:
        for bb in f.blocks:
            out = []
            for inst in bb.instructions:
                si = inst.sync_info
                if si is not None and len(si.on_wait) > max_waits:
                    waits = list(si.on_wait)
                    extra, keep = waits[:-max_waits], waits[-max_waits:]
                    for i in range(0, len(extra), max_waits):
                        nop = mybir.InstNoOp(
                            name=nc.get_next_instruction_name(),
                            engine=inst.engine,
                            sync_info=mybir.SyncInfo(
                                on_wait=extra[i : i + max_waits], on_update=[]
                            ),
                        )
                        out.append(nop)
                    si.on_wait = keep
                out.append(inst)
            bb.instructions.clear()
            for i in out:
                bb.add_instruction(i)


def _build_nc():
    import concourse.bass as bass
    import concourse.mybir as mybir
    from concourse.tile import TileContext

    fp32 = mybir.dt.float32
    bf16 = mybir.dt.bfloat16
    RELU = mybir.ActivationFunctionType.Relu
    ADD = mybir.AluOpType.add
    MAX = mybir.AluOpType.max

    nc = bass.Bass(trn_type="TRN2", num_devices=N_CORES)
    c4_d = nc.dram_tensor("c4", [NPAIR, 128, 4096], bf16, kind="ExternalInput")
    ctop_d = nc.dram_tensor("ctop", [128, 2048], bf16, kind="ExternalInput")
    wu_d = nc.dram_tensor("wu", [128, 512], bf16, kind="ExternalInput")
    whl_d = nc.dram_tensor("whl", [128, 128], bf16, kind="ExternalInput")
    whr_d = nc.dram_tensor("whr", [128, 128], bf16, kind="ExternalInput")
    whu_d = nc.dram_tensor("whu", [128, 128], bf16, kind="ExternalInput")
    bh2_d = nc.dram_tensor("bh2", [128, 1], fp32, kind="ExternalInput")
    out_d = nc.dram_tensor("out", [128, NPAIR], fp32, kind="ExternalOutput")

    # greedy act-engine balancing: est busy-ns per (scalar, vector)
    act_est = [0.0, 0.0]

    with TileContext(nc) as tc:
        with (
            tc.tile_pool(name="wpool", bufs=1) as wpool,
            tc.tile_pool(name="c4pool", bufs=3) as c4pool,
            tc.tile_pool(name="uspool", bufs=2) as uspool,
            tc.tile_pool(name="utpool", bufs=1) as utpool,
            tc.tile_pool(name="e12pool", bufs=2) as e12pool,
            tc.tile_pool(name="e11pool", bufs=2) as e11pool,
            tc.tile_pool(name="shpool", bufs=1) as shpool,
            tc.tile_pool(name="pspool", bufs=4, space="PSUM") as pspool,
        ):
            wu_sb = wpool.tile([128, 512], bf16, tag="wu")
            whl_sb = wpool.tile([128, 128], bf16, tag="whl")
            whr_sb = wpool.tile([128, 128], bf16, tag="whr")
            whu_sb = wpool.tile([128, 128], bf16, tag="whu")
            bh_sb = wpool.tile([128, 1], fp32, tag="bh")
            ctop_sb = wpool.tile([128, 2048], bf16, tag="ctop")
            c4_sbs = [None] * NPAIR

            def dma_c4(p, engine=None):
                c4_sbs[p] = c4pool.tile([128, 4096], bf16, tag="c4", name=f"c4_{p}")
                (engine or nc.sync).dma_start(c4_sbs[p][:], c4_d.ap()[p])

            # Head is DMA-*trigger* bound (~0.6us per trigger, serialized per
            # engine queue): spread the head triggers across four engine
            # queues so transfers start in parallel right after the preamble.
            # Pair 0's c4 is three separate tiles so the earliest groups can
            # start while the rest streams.
            c4_0aa = c4pool.tile([128, 1024], bf16, tag="c4q1", name="c4_0aa", bufs=1)
            c4_0ab = c4pool.tile([128, 1024], bf16, tag="c4q2", name="c4_0ab", bufs=1)
            c4_0b = c4pool.tile([128, 2048], bf16, tag="c4h2", name="c4_0b", bufs=1)
            nc.sync.dma_start(c4_0aa[:], c4_d.ap()[0][:, 0:1024])
            nc.scalar.dma_start(wu_sb[:], wu_d.ap())
            nc.scalar.dma_start(c4_0ab[:], c4_d.ap()[0][:, 1024:2048])
            nc.sync.dma_start(c4_0b[:], c4_d.ap()[0][:, 2048:4096])
            nc.gpsimd.dma_start(whl_sb[:], whl_d.ap())
            nc.gpsimd.dma_start(whr_sb[:], whr_d.ap())
            nc.gpsimd.dma_start(whu_sb[:], whu_d.ap())
            nc.gpsimd.dma_start(bh_sb[:], bh2_d.ap())
            dma_c4(1, engine=nc.gpsimd)
            nc.gpsimd.dma_start(ctop_sb[:], ctop_d.ap())

            def act_relu(dst_ap, src_ap, cols, bias):
                """relu(src [+ bias]) -> dst on the least-loaded act engine."""
                cs = act_est[0] + 120.0 + 0.95 * cols
                cv = act_est[1] + 150.0 + 1.06 * cols
                if cs <= cv:
                    act_est[0] = cs
                    if bias is None:
                        nc.scalar.activation(dst_ap, src_ap, RELU)
                    else:
                        nc.scalar.activation(dst_ap, src_ap, RELU, bias=bias)
                else:
                    act_est[1] = cv
                    if bias is None:
                        nc.vector.tensor_scalar(dst_ap, src_ap, 0.0, None, MAX)
                    else:
                        nc.vector.tensor_scalar(dst_ap, src_ap, bias, 0.0, ADD, MAX)

            def u_units(src_of, dst_tile, pname, order):
                """One thunk per fc_u group (1024 cols: 2 matmuls + act).
                src_of: g -> (tile, col_base) for that group's columns."""

                def mk(g):
                    def emit():
                        t = g % 4
                        src_sb, cc = src_of(g)
                        ps = pspool.tile(
                            [128, 1024], fp32, tag="ps", name=f"ups_{pname}_{g}"
                        )
                        for h in range(2):
                            nc.tensor.matmul(
                                ps[:, 512 * h : 512 * (h + 1)],
                                wu_sb[:, 128 * t : 128 * (t + 1)],
                                src_sb[:, cc + 512 * h : cc + 512 * (h + 1)],
                                start=True,
                                stop=True,
                            )
                        act_relu(
                            dst_tile[:, 1024 * g : 1024 * g + 1024],
                            ps[:, 0:1024],
                            1024,
                            None,
                        )

                    return emit

                return [mk(g) for g in order]

            def h_tile(prev, prev_base, u_ap, u_base, dst, dst_base, w, bname):
                """One fc_h psum tile (w<=1024 outputs): L,L,R,R,U,U then act."""
                ps = pspool.tile([128, 1024], fp32, tag="ps", name=f"hps_{bname}")
                for w_sb, kind in ((whl_sb, "L"), (whr_sb, "R"), (whu_sb, "U")):
                    for h0 in range(0, w, 512):
                        n = min(512, w - h0)
                        if kind == "L":
                            mv = prev[
                                :,
                                prev_base + 2 * h0 : prev_base + 2 * h0 + 2 * n : 2,
                            ]
                        elif kind == "R":
                            mv = prev[
                                :,
                                prev_base
                                + 2 * h0
                                + 1 : prev_base
                                + 2 * h0
                                + 2 * n : 2,
                            ]
                        else:
                            mv = u_ap[:, u_base + h0 : u_base + h0 + n]
                        nc.tensor.matmul(
                            ps[:, h0 : h0 + n],
                            w_sb[:],
                            mv,
                            start=(kind == "L"),
                            stop=(kind == "U"),
                        )
                act_relu(
                    dst[:, dst_base : dst_base + w], ps[:, 0:w], w, bh_sb[:]
                )

            def h_block(prev, prev_base, u_ap, u_base, dst, dst_base, ncols, bname):
                """One fc_h stretch as a sequence of 1024-col tiles."""
                for c0 in range(0, ncols, 1024):
                    w = min(1024, ncols - c0)
                    h_tile(
                        prev,
                        prev_base + 2 * c0,
                        u_ap,
                        u_base + c0,
                        dst,
                        dst_base + c0,
                        w,
                        f"{bname}_{c0}",
                    )

            emb10sh = shpool.tile([128, 8192], bf16, tag="e10")
            ustrs = [None] * NPAIR

            def h_units(p):
                """Per-tile thunks for pair p's levels 12..10 (7 tiles)."""
                ustr = ustrs[p]
                emb12 = e12pool.tile([128, 4096], bf16, tag="e12", name=f"e12_{p}")
                emb11 = e11pool.tile([128, 2048], bf16, tag="e11", name=f"e11_{p}")
                units = []
                for c in range(4):
                    units.append(
                        lambda c=c: h_tile(
                            ustr,
                            ULEAF + 2048 * c,
                            ustr,
                            UB12 + 1024 * c,
                            emb12,
                            1024 * c,
                            1024,
                            f"l12_{p}_{c}",
                        )
                    )
                for c in range(2):
                    units.append(
                        lambda c=c: h_tile(
                            emb12,
                            2048 * c,
                            ustr,
                            UB11 + 1024 * c,
                            emb11,
                            1024 * c,
                            1024,
                            f"l11_{p}_{c}",
                        )
                    )
                units.append(
                    lambda: h_tile(
                        emb11, 0, ustr, UB10, emb10sh, 1024 * p, 1024, f"l10_{p}"
                    )
                )
                return units

            # ---- blended body: u-tiles of pair p interleaved ~2:1 with
            # h-tiles of pair p-1 (whose inputs are fully materialized), so
            # the act engines see a demand below their combined supply and
            # the PE never waits on psum recycling. ----
            utop = utpool.tile([128, UTOP_PAD], bf16, tag="utop")
            # pair 0 consumes its two c4 half-tiles in landing order; other
            # pairs strip-major (level-12 u groups early).
            order_p0 = [0, 1, 2, 3, 4, 5, 6, 7, 8, 12, 9, 13, 10, 14, 11]

            def src_p0(g):
                if g <= 3:
                    return c4_0aa, 0
                if g <= 7:
                    return c4_0ab, 0
                return c4_0b, 1024 * (g // 4 - 2)

            order_rest = [3, 7, 11, 0, 4, 8, 12, 1, 5, 9, 13, 2, 6, 10, 14]
            pend_h = []
            for p in range(NPAIR + 1):
                if p < NPAIR:
                    if p + 2 < NPAIR:
                        dma_c4(p + 2)
                    ustrs[p] = uspool.tile(
                        [128, USTREAM], bf16, tag="us", name=f"ustr{p}"
                    )
                    if p == 0:
                        uu = u_units(src_p0, ustrs[0], "p0", order_p0)
                    else:
                        src = c4_sbs[p]
                        uu = u_units(
                            lambda g, s=src: (s, 1024 * (g // 4)),
                            ustrs[p],
                            f"p{p}",
                            order_rest,
                        )
                    nu = 2  # 15 u : 7 h per pair ~ 2:1
                else:
                    # final round: u for levels 0..9 (utop) blends 1:1 with
                    # pair 7's h-tiles; level-9/root input groups first
                    uu = u_units(
                        lambda g: (ctop_sb, 1024 * (g // 4)),
                        utop,
                        "top",
                        [3, 7, 4, 0, 5, 1, 6, 2],
                    )
                    nu = 1
                ui = hi = 0
                while ui < len(uu) or hi < len(pend_h):
                    for _ in range(nu):
                        if ui < len(uu):
                            uu[ui]()
                            ui += 1
                    if hi < len(pend_h):
                        pend_h[hi]()
                        hi += 1
                pend_h = h_units(p) if p < NPAIR else []

            # ---- levels 9..1 batched; two 4-pair chains down to level 6,
            # single chain below (chain-splitting tiny levels just adds
            # instruction overhead) ----
            prev = emb10sh
            emb_sh = {}
            for i in range(9, 0, -1):
                m = 2**i
                M8 = 8 * m
                cur = shpool.tile([128, M8], bf16, tag=f"esh{i}")
                emb_sh[i] = cur
                base8 = 8 * (2**i - 1)
                nchain = 2 if i >= 2 else 1
                half = M8 // nchain
                for chain in range(nchain):
                    h_block(
                        prev,
                        chain * 2 * half,
                        utop,
                        base8 + chain * half,
                        cur,
                        chain * half,
                        half,
                        f"b{i}_{chain}",
                    )
                prev = cur

            # ---- level 0: root ----
            roots = wpool.tile([128, NPAIR], fp32, tag="roots")
            ps = pspool.tile([128, 1024], fp32, tag="ps", name="root_ps")
            o = ps[:, 0:NPAIR]
            nc.tensor.matmul(o, whl_sb[:], emb_sh[1][:, 0:16:2], start=True, stop=False)
            nc.tensor.matmul(o, whr_sb[:], emb_sh[1][:, 1:16:2], start=False, stop=False)
            nc.tensor.matmul(o, whu_sb[:], utop[:, 0:NPAIR], start=False, stop=True)
            nc.scalar.activation(roots[:], o, RELU, bias=bh_sb[:])
            nc.sync.dma_start(out_d.ap(), roots[:])

    _dedup_ldweights(nc)
    _split_sync_waits(nc, mybir)
    return nc


_NC_CACHE = None
LAST_RESULTS = None


def kernel(contents, children, Wu, bu, Wh, bh):
    global _NC_CACHE, LAST_RESULTS
    contents = np.asarray(contents, dtype=np.float32)
    children = np.asarray(children)
    Wu = np.asarray(Wu, dtype=np.float32)
    bu = np.asarray(bu, dtype=np.float32)
    Wh = np.asarray(Wh, dtype=np.float32)
    bh = np.asarray(bh, dtype=np.float32)

    regular = (
        contents.shape == (B, N_NODES, F)
        and children.shape == (N_INNER, 2)
        and np.array_equal(
            np.asarray(children, dtype=np.int64).ravel(), np.arange(N_INNER * 2)
        )
    )
    if not regular:
        # Safety net for non-arange children: exact numpy fallback.
        return _np_reference(contents, children, Wu, bu, Wh, bh)

    from concourse.bass_utils import run_bass_kernel_spmd

    if _NC_CACHE is None:
        _NC_CACHE = _build_nc()
    nc = _NC_CACHE

    wts = _prep_weights(Wu, bu, Wh, bh)
    in_maps = []
    for k in range(N_CORES):
        m = _prep_core_inputs(contents[JPC * k : JPC * (k + 1)])
        m.update(wts)
        in_maps.append(m)

    res = run_bass_kernel_spmd(
        nc,
        in_maps,
        core_ids=list(range(N_CORES)),
        trace=bool(os.environ.get("BASS_TRACE")),
    )
    LAST_RESULTS = res

    out = np.empty((B, H), dtype=np.float32)
    for k in range(N_CORES):
        r = res.results[k]["out"].reshape(2, 64, NPAIR)  # [half, h, pair]
        out[JPC * k : JPC * (k + 1)] = np.transpose(r, (2, 0, 1)).reshape(JPC, H)
    return out
